# revision 1
# baseline (speedup 1.0000x reference)
"""Trainium2 Bass kernel for nn_CondScoreModelGNN (8-core SPMD).

Graph structure exploited: dst = tile(arange(N), 10) -> every node receives
exactly 10 edges at stride N; segment_max becomes a regular blocked max.

Sharding: nodes (padded 50000->50176) split into 8 contiguous shards of 6272.
Each core computes its shard's node features feature-major ([feat, node]),
builds its shard of the EdgeConv neighbor table (hn = x @ Wb) row-major,
AllGathers the table across cores, then processes its shard's 62720 edges:
indirect-DMA row gathers + PE transpose + add/relu + PE matmul (e @ W2) +
max accumulation. MLP weights are replicated.
"""
import sys

sys.path.insert(0, "/opt/trn_rl_repo")

import numpy as np

N_CORES = 8
N = 50000
E = 500000
B = 1024
H = 128
EM = 64
CLS = 10
KDEG = 10          # edges per node
NPAD = 50176       # 8 * 6272
SH = 6272          # nodes per core
NBLK = 49          # 128-node blocks per core
TWO_PI = 2.0 * np.pi
SIGMA = 25.0
LOG_SIGMA = float(np.log(SIGMA))

# node chunks of 512 (12 full + 1 tail of 128)
CHUNKS = [(i * 512, 512) for i in range(12)] + [(6144, 128)]
NCH = len(CHUNKS)

_CACHE = {}


def _split_multi_waits(nc, mybir):
    """This walrus build encodes at most one sync wait per TPB_CTRL
    instruction; hoist extra waits into single-wait EventSemaphore insts."""
    n_split = 0
    for fn in nc.m.functions:
        for bb in fn.blocks:
            insts = list(bb.instructions)
            out = []
            changed = False
            for ins in insts:
                si = ins.sync_info
                waits = list(si.on_wait) if (si is not None and si.on_wait) else []
                is_drain = type(ins).__name__ == "InstDrain"
                if (len(waits) > 1) or (is_drain and len(waits) > 0):
                    changed = True
                    n_split += 1
                    for w in waits:
                        ev = mybir.InstEventSemaphore(
                            name=nc.get_next_instruction_name(),
                            opcode="EventSemaphore",
                            engine=ins.engine,
                            ins=[],
                            outs=[],
                            sync_info=mybir.SyncInfo(on_wait=[w], on_update=[]),
                        )
                        nc.register_instruction(ev)
                        out.append(ev)
                    si.on_wait = []
                    ins.sync_info = si
                out.append(ins)
            if changed:
                bb.instructions = out
    return n_split


def _build(debug=False, iters=1):
    import concourse.bass as bass
    import concourse.tile as tile
    from concourse import mybir
    from concourse.masks import make_identity

    f32 = mybir.dt.float32
    f32r = mybir.dt.float32r
    i32 = mybir.dt.int32
    AF = mybir.ActivationFunctionType
    ALU = mybir.AluOpType

    nc = bass.Bass()

    # ---------------- I/O ----------------
    band_in = nc.dram_tensor("band_in", [128, SH], f32, kind="ExternalInput")
    srcidx = nc.dram_tensor("srcidx", [128, KDEG * NBLK], i32, kind="ExternalInput")
    smallw = nc.dram_tensor("smallw", [128, 256], f32, kind="ExternalInput")
    btile_d = nc.dram_tensor("btile", [128, 16], f32, kind="ExternalInput")
    wnames = [
        ("i2", [128, 128]), ("sW", [64, 64]), ("w2", [128, 64]),
        ("m1W2", [128, 128]), ("m2W2", [128, 4]),
        ("m1cA", [128, 128]), ("m1cB", [128, 128]), ("m1cC", [64, 128]),
        ("m1nA", [128, 128]), ("m1nB", [128, 128]), ("m1nC", [64, 128]),
        ("m2cA", [128, 128]), ("m2cB", [128, 128]), ("m2cC", [64, 128]),
        ("m2nA", [128, 128]), ("m2nB", [128, 128]), ("m2nC", [64, 128]),
    ]
    wdram = {n: nc.dram_tensor(n, s, f32, kind="ExternalInput") for n, s in wnames}
    y_out = nc.dram_tensor("y", [128, 1664], f32, kind="ExternalOutput")
    dbg = {}
    if debug:
        for nm, sh in [("dxT0", [128, SH]), ("dxT1", [128, SH]),
                       ("dxT2", [64, SH]), ("dhcb", [128, SH]),
                       ("dout1", [128, SH]), ("dtab", [1024, 128]),
                       ("dmsg", [128, 512]), ("dgd", [128, 512]),
                       ("dacc2", [128, 1664]), ("dband", [128, SH])]:
            dbg[nm] = nc.dram_tensor(nm, sh, f32, kind="ExternalOutput")

    # collective bounce buffers
    inb = [nc.dram_tensor(f"inb{v}", [SH, 128], f32) for v in range(2)]
    outb = [
        nc.dram_tensor(f"outb{v}", [NPAD, 128], f32, addr_space="Shared")
        for v in range(2)
    ]

    RG = [list(range(N_CORES))]

    with tile.TileContext(nc) as tc:
        with (
            tc.tile_pool(name="wpool", bufs=1) as wpool,
            tc.tile_pool(name="npool", bufs=1) as npool,
            tc.tile_pool(name="psA", bufs=4, space="PSUM") as psA,
            tc.tile_pool(name="psT", bufs=2, space="PSUM") as psT,
            tc.tile_pool(name="psO", bufs=2, space="PSUM") as psO,
        ):
            # ---------- persistent SBUF ----------
            band = npool.tile([128, SH], f32)       # xin@[0:6] oh@[32:42] t@[64:65] wall@[96:98]; recip->[64:65]
            xT0 = npool.tile([128, SH], f32)        # init (conv1) -> later reused pattern kept separate
            xT1 = npool.tile([128, SH], f32)        # class(0:64) + sigma(64:128)
            xT2 = npool.tile([64, SH], f32)         # wall_feat
            hcb = npool.tile([128, SH], f32)        # center term + b1 (per conv)
            acc = npool.tile([128, SH], f32)        # conv1 max accum -> out1 (= conv2's xT0)
            acc2v = npool.tile([128, 1664], f32)    # conv2 packed accum
            yv = npool.tile([128, 1664], f32)
            sidx = npool.tile([128, KDEG * NBLK], i32)

            smw = wpool.tile([128, 256], f32)
            bt = wpool.tile([128, 16], f32)
            ident = wpool.tile([128, 128], f32)
            wt = {n: wpool.tile(s, f32, name=f"wt_{n}") for n, s in wnames}

            nc.sync.dma_start(out=band[:], in_=band_in[:, :])
            nc.sync.dma_start(out=sidx[:], in_=srcidx[:, :])
            nc.sync.dma_start(out=smw[:], in_=smallw[:, :])
            nc.sync.dma_start(out=bt[:], in_=btile_d[:, :])
            for n, s in wnames:
                nc.sync.dma_start(out=wt[n][:], in_=wdram[n][:, :])
            make_identity(nc, ident[:])

            def mm(out_ap, lhsT_ap, rhs_ap, start=True, stop=True,
                   tile_position=None):
                nc.tensor.matmul(
                    out_ap,
                    lhsT_ap,
                    rhs_ap,
                    start=start,
                    stop=stop,
                    tile_position=tile_position,
                )

            # bias column APs
            ib1 = bt[:, 0:1]
            ib2 = bt[:, 1:2]
            b_cs = bt[:, 2:3]        # [0:64]=0 (class), [64:128]=sb
            wb2 = bt[0:64, 4:5]
            m1b1 = bt[:, 5:6]
            m1b2 = bt[:, 6:7]
            m2b1 = bt[:, 7:8]
            m2b2v = bt[:, 8:9]       # b2 replicated at 32q+i

            for it_ in range(iters):
                # ---------- phase A: node features ----------
                with (
                    tc.tile_pool(name=f"stg{it_}", bufs=2) as stg,
                ):
                    for cs, cw in CHUNKS:
                        csl = slice(cs, cs + cw)
                        # init MLP
                        p1 = psA.tile([128, 512], f32, tag="pa")
                        mm(p1[:, :cw], smw[0:6, 0:128], band[0:6, csl])
                        i1s = stg.tile([128, 512], f32, tag="i1s")
                        nc.scalar.activation(i1s[:, :cw], p1[:, :cw], AF.Relu, bias=ib1)
                        p2 = psA.tile([128, 512], f32, tag="pa")
                        mm(p2[:, :cw], wt["i2"][:], i1s[:, :cw])
                        nc.scalar.activation(xT0[:, csl], p2[:, :cw], AF.Relu, bias=ib2)

                        # block-diag mm at base 64: rows 64..67 = [t, wall0,
                        # wall1, ones]. out [0:32]=t*w+256 (sin arg), [32:64]=
                        # t*w+0.25 (cos arg), [64:128]=wall_h1 pre-act. Range-
                        # reduce m = x - round(x) via i32 cast (round-nearest),
                        # then sin(2*pi*m) = sin(2*pi*x).
                        p3 = psA.tile([128, 512], f32, tag="pa")
                        mm(p3[:, :cw], smw[64:68, 0:128], band[64:68, csl])
                        gfpi = stg.tile([64, 512], i32, tag="gfpi")
                        nc.vector.tensor_copy(out=gfpi[0:64, :cw], in_=p3[0:64, :cw])
                        gfpf = stg.tile([64, 512], f32, tag="gfpf")
                        nc.vector.tensor_copy(out=gfpf[0:64, :cw], in_=gfpi[0:64, :cw])
                        gfpm = stg.tile([64, 512], f32, tag="gfpm")
                        nc.vector.scalar_tensor_tensor(
                            out=gfpm[0:64, :cw], in0=p3[0:64, :cw], scalar=0.0,
                            in1=gfpf[0:64, :cw], op0=ALU.bypass, op1=ALU.subtract)
                        gfps = stg.tile([64, 512], f32, tag="gfps")
                        nc.scalar.activation(gfps[0:64, :cw], gfpm[0:64, :cw], AF.Sin,
                                             scale=TWO_PI)
                        whs = stg.tile([128, 512], f32, tag="whs")
                        nc.scalar.activation(whs[64:128, :cw], p3[64:128, :cw], AF.Relu,
                                             bias=bt[64:128, 3:4])
                        # class (psum [0:64]) + sigma (psum [64:128])
                        p4 = psA.tile([128, 512], f32, tag="pa")
                        mm(p4[0:64, :cw], smw[32:42, 0:64], band[32:42, csl])
                        mm(p4[64:128, :cw], wt["sW"][:], gfps[:, :cw])
                        nc.scalar.activation(xT1[:, csl], p4[:, :cw], AF.Relu, bias=b_cs)

                        # wall MLP layer 2
                        p6 = psA.tile([128, 512], f32, tag="pa")
                        mm(p6[0:64, :cw], wt["w2"][64:128, :], whs[64:128, :cw])
                        nc.scalar.activation(xT2[:, csl], p6[0:64, :cw], AF.Relu, bias=wb2)

                        # std(t): exp -> (x-1)/(2 ln s) -> sqrt -> +eps -> recip
                        # all on partition 64 (lane-aligned)
                        sstd = stg.tile([128, 1024], f32, tag="sstd")
                        nc.scalar.activation(sstd[64:65, 0:cw], band[64:65, csl], AF.Exp,
                                             scale=2.0 * LOG_SIGMA)
                        nc.vector.tensor_scalar(
                            out=sstd[64:65, 512:512 + cw], in0=sstd[64:65, 0:cw],
                            scalar1=-1.0, scalar2=1.0 / (2.0 * LOG_SIGMA),
                            op0=ALU.add, op1=ALU.mult)
                        nc.scalar.activation(sstd[64:65, 0:cw],
                                             sstd[64:65, 512:512 + cw], AF.Sqrt)
                        nc.vector.tensor_scalar_add(
                            out=sstd[64:65, 512:512 + cw], in0=sstd[64:65, 0:cw],
                            scalar1=1e-7)
                        nc.vector.reciprocal(out=band[64:65, csl],
                                             in_=sstd[64:65, 512:512 + cw])

                nc.gpsimd.memset(acc[:], -3.0e38)
                nc.gpsimd.memset(acc2v[:], -3.0e38)

                # ---------- per-conv helpers ----------
                def x_mm(out_ap, wa, wb, wc, csl, cw, x0):
                    """out = W.T @ x for one chunk; x = [x0(128); xT1(128); xT2(64)]"""
                    mm(out_ap, wt[wa][:], x0[:, csl], start=True, stop=False)
                    mm(out_ap, wt[wb][:], xT1[:, csl], start=False, stop=False)
                    mm(out_ap, wt[wc][:], xT2[:, csl], start=False, stop=True)

                def build_table(conv, x0):
                    """hn = x @ Wb -> transpose -> DRAM shard -> AllGather."""
                    wa, wb, wc = (f"m{conv}nA", f"m{conv}nB", f"m{conv}nC")
                    with (
                        tc.tile_pool(name=f"hnst{conv}_{it_}", bufs=2) as hnp,
                        tc.tile_pool(name=f"rwst{conv}_{it_}", bufs=2) as rwp,
                    ):
                        for ci, (cs, cw) in enumerate(CHUNKS):
                            csl = slice(cs, cs + cw)
                            ph = psA.tile([128, 512], f32, tag="pa")
                            x_mm(ph[:, :cw], wa, wb, wc, csl, cw, x0)
                            hns = hnp.tile([128, 512], f32, tag="hns")
                            nc.scalar.copy(hns[:, :cw], ph[:, :cw])
                            nb = cw // 128
                            pt = psT.tile([128, 512], f32, tag="pt")
                            for b in range(nb):
                                nc.tensor.transpose(
                                    out=pt[:, 128 * b:128 * (b + 1)],
                                    in_=hns[:, 128 * b:128 * (b + 1)],
                                    identity=ident[:],
                                )
                            rws = rwp.tile([128, 512], f32, tag="rws")
                            nc.vector.tensor_copy(out=rws[:, :cw], in_=pt[:, :cw])
                            dst_ap = inb[conv - 1][csl, :].rearrange(
                                "(b p) f -> p b f", p=128)
                            src_ap = rws[:].rearrange("p (b f) -> p b f", f=128)[:, :nb, :]
                            nc.sync.dma_start(out=dst_ap, in_=src_ap)
                    nc.gpsimd.collective_compute(
                        "AllGather", ALU.bypass, replica_groups=RG,
                        ins=[inb[conv - 1].ap().opt()],
                        outs=[outb[conv - 1].ap().opt()],
                    )

                def build_hcb(conv, x0):
                    wa, wb, wc = (f"m{conv}cA", f"m{conv}cB", f"m{conv}cC")
                    b1 = m1b1 if conv == 1 else m2b1
                    for cs, cw in CHUNKS:
                        csl = slice(cs, cs + cw)
                        ph = psA.tile([128, 512], f32, tag="pa")
                        x_mm(ph[:, :cw], wa, wb, wc, csl, cw, x0)
                        nc.scalar.activation(hcb[:, csl], ph[:, :cw], AF.Identity,
                                             bias=b1)

                def edge_pipeline(conv):
                    tab = outb[conv - 1]
                    with (
                        tc.tile_pool(name=f"gd{conv}_{it_}", bufs=3) as gdp,
                        tc.tile_pool(name=f"ms{conv}_{it_}", bufs=3) as msp,
                    ):
                        for r in range(KDEG):
                            if conv == 1:
                                for ci, (cs, cw) in enumerate(CHUNKS):
                                    csl = slice(cs, cs + cw)
                                    nb = cw // 128
                                    m = _edge_msgs(tab, gdp, msp, r, ci, cs, cw, nb)
                                    po = psO.tile([128, 512], f32, tag="po")
                                    mm(po[:, :cw], wt["m1W2"][:], m[:, :cw])
                                    nc.vector.scalar_tensor_tensor(
                                        out=acc[:, csl], in0=po[:, :cw],
                                        scalar=m1b2, in1=acc[:, csl],
                                        op0=ALU.add, op1=ALU.max)
                            else:
                                for G in range(4):
                                    ccs = list(range(4 * G, min(4 * G + 4, NCH)))
                                    po = psO.tile([128, 512], f32, tag="po")
                                    for qd, ci in enumerate(ccs):
                                        cs, cw = CHUNKS[ci]
                                        nb = cw // 128
                                        m = _edge_msgs(tab, gdp, msp, r, ci, cs, cw, nb)
                                        mm(po[32 * qd:32 * qd + 4, :cw],
                                           wt["m2W2"][:], m[:, :cw],
                                           tile_position=(0, 32 * qd))
                                    if G < 3:
                                        nc.vector.scalar_tensor_tensor(
                                            out=acc2v[:, 512 * G:512 * (G + 1)],
                                            in0=po[:, :512], scalar=m2b2v,
                                            in1=acc2v[:, 512 * G:512 * (G + 1)],
                                            op0=ALU.add, op1=ALU.max)
                                    else:
                                        nc.vector.scalar_tensor_tensor(
                                            out=acc2v[0:4, 1536:1664],
                                            in0=po[0:4, :128], scalar=bt[0:4, 8:9],
                                            in1=acc2v[0:4, 1536:1664],
                                            op0=ALU.add, op1=ALU.max)

                dumped = {"done": False}

                def _edge_msgs(tab, gdp, msp, r, ci, cs, cw, nb):
                    gd = gdp.tile([128, 4, 128], f32, tag="gd")
                    for b in range(nb):
                        q = NBLK * r + 4 * ci + b
                        nc.gpsimd.indirect_dma_start(
                            out=gd[:, b, :],
                            out_offset=None,
                            in_=tab[:, :],
                            in_offset=bass.IndirectOffsetOnAxis(
                                ap=sidx[:, q:q + 1], axis=0),
                        )
                    pt = psT.tile([128, 512], f32, tag="pt")
                    for b in range(nb):
                        nc.tensor.transpose(
                            out=pt[:, 128 * b:128 * (b + 1)],
                            in_=gd[:, b, :],
                            identity=ident[:],
                        )
                    m = msp.tile([128, 512], f32, tag="m")
                    nc.vector.tensor_add(
                        out=m[:, :cw], in0=pt[:, :cw],
                        in1=hcb[:, cs:cs + cw])
                    nc.scalar.activation(m[:, :cw], m[:, :cw], AF.Relu)
                    if debug and not dumped["done"]:
                        dumped["done"] = True
                        nc.sync.dma_start(
                            out=dbg["dgd"][:, :],
                            in_=gd[:].rearrange("p a b -> p (a b)"))
                        nc.sync.dma_start(out=dbg["dmsg"][:, :], in_=m[:])
                    return m

                # ---------- conv1 ----------
                build_table(1, xT0)
                build_hcb(1, xT0)
                edge_pipeline(1)
                # out1 = relu(acc) in place
                for cs, cw in CHUNKS:
                    csl = slice(cs, cs + cw)
                    nc.scalar.activation(acc[:, csl], acc[:, csl], AF.Relu)

                if debug:
                    nc.sync.dma_start(out=dbg["dxT0"][:, :], in_=xT0[:])
                    nc.sync.dma_start(out=dbg["dxT1"][:, :], in_=xT1[:])
                    nc.sync.dma_start(out=dbg["dxT2"][:, :], in_=xT2[:])
                    nc.sync.dma_start(out=dbg["dhcb"][:, :], in_=hcb[:])
                    nc.sync.dma_start(out=dbg["dout1"][:, :], in_=acc[:])
                    nc.sync.dma_start(out=dbg["dtab"][:, :],
                                      in_=outb[0][25000:26024, :])
                    nc.sync.dma_start(out=dbg["dband"][:, :], in_=band[:])

                # ---------- conv2 ----------
                build_table(2, acc)
                build_hcb(2, acc)
                edge_pipeline(2)

                if debug:
                    nc.sync.dma_start(out=dbg["dacc2"][:, :], in_=acc2v[:])

                # ---------- final scale: y = acc2 * recip ----------
                for G in range(4):
                    ccs = list(range(4 * G, min(4 * G + 4, NCH)))
                    pr = psA.tile([128, 512], f32, tag="pa")
                    gw = 512 if G < 3 else 128
                    for qd, ci in enumerate(ccs):
                        cs, cw = CHUNKS[ci]
                        mm(pr[32 * qd:32 * qd + 4, :cw],
                           smw[64:65, 132:136], band[64:65, cs:cs + cw],
                           tile_position=(64, 32 * qd))
                    gcol = slice(512 * G, 512 * G + gw)
                    if G < 3:
                        nc.vector.tensor_tensor(
                            out=yv[:, gcol], in0=acc2v[:, gcol], in1=pr[:, :gw],
                            op=ALU.mult)
                    else:
                        nc.vector.tensor_tensor(
                            out=yv[0:4, gcol], in0=acc2v[0:4, gcol],
                            in1=pr[0:4, :gw], op=ALU.mult)
                nc.sync.dma_start(out=y_out[:, :], in_=yv[:])

    _split_multi_waits(nc, mybir)
    return nc


def _host_prep(inputs):
    """Build per-core input maps from full inputs."""
    t = np.asarray(inputs["t"], np.float32).reshape(N)
    obj_x = np.asarray(inputs["obj_x"], np.float32)
    obj_geo = np.asarray(inputs["obj_geo"], np.float32)
    wall = np.asarray(inputs["wall"], np.float32)
    category = np.asarray(inputs["category"]).astype(np.int64)
    batch_idx = np.asarray(inputs["batch_idx"]).astype(np.int64)
    src = np.asarray(inputs["src"]).astype(np.int64)
    dst = np.asarray(inputs["dst"]).astype(np.int64)

    # edge slots: slot (r, n) holds the r-th incoming edge of node n
    if np.array_equal(dst, np.tile(np.arange(N, dtype=dst.dtype), E // N)):
        src_slots = src.reshape(KDEG, N)
    else:
        order = np.argsort(dst, kind="stable")
        counts = np.bincount(dst, minlength=N)
        assert (counts == KDEG).all(), "kernel requires uniform in-degree 10"
        src_slots = np.empty((KDEG, N), np.int64)
        srt = src[order].reshape(N, KDEG)
        src_slots[:, :] = srt.T

    wall_pn = wall[batch_idx]  # [N, 2]

    # weights (shared)
    def f32c(x):
        return np.ascontiguousarray(np.asarray(x, np.float32))

    smallw = np.zeros((128, 256), np.float32)
    smallw[0:6, 0:128] = f32c(inputs["i1"])
    smallw[32:42, 0:64] = f32c(inputs["embed_W"])
    gw = f32c(inputs["gfp_W"]).reshape(32)
    smallw[64, 0:32] = gw
    smallw[64, 32:64] = gw
    smallw[65:67, 64:128] = f32c(inputs["w1"])   # wall layer 1 block
    smallw[67, 32:64] = 0.25      # cos = sin(2*pi*(x + 1/4))
    smallw[64, 132:136] = 1.0  # ones4 for recip broadcast

    btile = np.zeros((128, 16), np.float32)
    btile[:, 0] = f32c(inputs["ib1"])
    btile[:, 1] = f32c(inputs["ib2"])
    btile[64:128, 2] = f32c(inputs["sb"])
    btile[64:128, 3] = f32c(inputs["wb1"])
    btile[0:64, 4] = f32c(inputs["wb2"])
    btile[:, 5] = f32c(inputs["m1b1"])
    btile[:, 6] = f32c(inputs["m1b2"])
    btile[:, 7] = f32c(inputs["m2b1"])
    b2 = f32c(inputs["m2b2"])
    for q in range(4):
        btile[32 * q:32 * q + 4, 8] = b2
    btile[:, 9] = np.float32(np.pi / 2.0)
    btile[:, 10] = np.float32(-np.pi)

    m1W1 = f32c(inputs["m1W1"])
    m2W1 = f32c(inputs["m2W1"])
    m1c = m1W1[:320] - m1W1[320:]
    m1n = m1W1[320:]
    m2c = m2W1[:320] - m2W1[320:]
    m2n = m2W1[320:]

    wmap = {
        "i2": f32c(inputs["i2"]), "sW": f32c(inputs["sW"]),
        "w2": np.concatenate(
            [np.zeros((64, 64), np.float32), f32c(inputs["w2"])], axis=0),
        "m1W2": f32c(inputs["m1W2"]), "m2W2": f32c(inputs["m2W2"]),
        "m1cA": m1c[0:128], "m1cB": m1c[128:256], "m1cC": m1c[256:320],
        "m1nA": m1n[0:128], "m1nB": m1n[128:256], "m1nC": m1n[256:320],
        "m2cA": m2c[0:128], "m2cB": m2c[128:256], "m2cC": m2c[256:320],
        "m2nA": m2n[0:128], "m2nB": m2n[128:256], "m2nC": m2n[256:320],
    }
    wmap = {k: np.ascontiguousarray(v) for k, v in wmap.items()}

    in_maps = []
    for c in range(N_CORES):
        n0 = c * SH
        nreal = min(max(N - n0, 0), SH)
        band = np.zeros((128, SH), np.float32)
        band[0:4, :nreal] = obj_x[n0:n0 + nreal].T
        band[4:6, :nreal] = obj_geo[n0:n0 + nreal].T
        cat = category[n0:n0 + nreal]
        band[32 + cat, np.arange(nreal)] = 1.0  # one-hot
        band[64, :nreal] = t[n0:n0 + nreal]
        band[64, nreal:] = 1.0
        band[65:67, :nreal] = wall_pn[n0:n0 + nreal].T
        band[67, :] = 1.0

        sidx = np.zeros((128, KDEG * NBLK), np.int32)
        sl = src_slots[:, n0:n0 + nreal]  # [10, nreal]
        sfull = np.zeros((KDEG, SH), np.int64)
        sfull[:, :nreal] = sl
        # column q = 49*r + j holds nodes 128j..128j+127 of round r
        sidx[:, :] = (
            sfull.reshape(KDEG, NBLK, 128).transpose(2, 0, 1).reshape(
                128, KDEG * NBLK)
        ).astype(np.int32)

        im = {
            "band_in": band,
            "srcidx": sidx,
            "smallw": smallw,
            "btile": btile,
        }
        im.update(wmap)
        in_maps.append(im)
    return in_maps


def _unshard(results):
    out = np.empty((NPAD, 4), np.float32)
    for c in range(N_CORES):
        yv = results[c]["y"]  # [128, 1664]
        for cc in range(NCH):
            G, q = cc // 4, cc % 4
            cs, cw = CHUNKS[cc]
            out[c * SH + cs: c * SH + cs + cw, :] = (
                yv[32 * q:32 * q + 4, 512 * G:512 * G + cw].T
            )
    return out[:N]


def kernel(**inputs) -> np.ndarray:
    from concourse.bass_utils import run_bass_kernel_spmd

    if "nc" not in _CACHE:
        _CACHE["nc"] = _build()
    nc = _CACHE["nc"]
    in_maps = _host_prep(inputs)
    import time as _time
    last_err = None
    for attempt in range(4):
        try:
            res = run_bass_kernel_spmd(nc, in_maps,
                                       core_ids=list(range(N_CORES)))
            break
        except Exception as e:  # transient NRT device wedge recovers on retry
            last_err = e
            _time.sleep(15 * (attempt + 1))
    else:
        raise last_err
    _CACHE["last_results"] = res
    return _unshard(res.results)


import concourse.bass as bass  # noqa: E402  (used inside _build closures)



# revision 25
# speedup vs baseline: 365.0340x; 365.0340x over previous
"""Trainium2 Bass kernel for nn_CondScoreModelGNN (8-core SPMD).

Graph structure exploited: dst = tile(arange(N), 10) -> every node receives
exactly 10 edges at stride N; segment_max becomes a regular blocked max.

Sharding: nodes (padded 50000->50176) split into 8 contiguous shards of 6272.
Each core computes its shard's node features feature-major ([feat, node]),
builds its shard of the EdgeConv neighbor table (hn = x @ Wb) row-major,
AllGathers the table across cores, then processes its shard's 62720 edges:
indirect-DMA row gathers + PE transpose + add/relu + PE matmul (e @ W2) +
max accumulation. MLP weights are replicated.
"""
import sys

sys.path.insert(0, "/opt/trn_rl_repo")

import numpy as np

N_CORES = 8
N = 50000
E = 500000
B = 1024
H = 128
EM = 64
CLS = 10
KDEG = 10          # edges per node
NPAD = 50176       # 8 * 6272
SH = 6272          # nodes per core
NBLK = 49          # 128-node blocks per core
TWO_PI = 2.0 * np.pi
SIGMA = 25.0
LOG_SIGMA = float(np.log(SIGMA))

# node chunks of 512 (12 full + 1 tail of 128)
CHUNKS = [(i * 512, 512) for i in range(12)] + [(6144, 128)]
NCH = len(CHUNKS)

_CACHE = {}


def _split_multi_waits(nc, mybir):
    """This walrus build encodes at most one sync wait per TPB_CTRL
    instruction; hoist extra waits into single-wait EventSemaphore insts."""
    n_split = 0
    for fn in nc.m.functions:
        for bb in fn.blocks:
            insts = list(bb.instructions)
            out = []
            changed = False
            for ins in insts:
                si = ins.sync_info
                waits = list(si.on_wait) if (si is not None and si.on_wait) else []
                is_drain = type(ins).__name__ == "InstDrain"
                if (len(waits) > 1) or (is_drain and len(waits) > 0):
                    changed = True
                    n_split += 1
                    for w in waits:
                        ev = mybir.InstEventSemaphore(
                            name=nc.get_next_instruction_name(),
                            opcode="EventSemaphore",
                            engine=ins.engine,
                            ins=[],
                            outs=[],
                            sync_info=mybir.SyncInfo(on_wait=[w], on_update=[]),
                        )
                        nc.register_instruction(ev)
                        out.append(ev)
                    si.on_wait = []
                    ins.sync_info = si
                out.append(ins)
            if changed:
                bb.instructions = out
    return n_split


def _build(debug=False, iters=1, ablate=frozenset()):
    ablate = frozenset(ablate)
    import concourse.bass as bass
    import concourse.tile as tile
    from concourse import mybir
    from concourse.masks import make_identity

    f32 = mybir.dt.float32
    f32r = mybir.dt.float32r
    bf16 = mybir.dt.bfloat16
    i32 = mybir.dt.int32
    AF = mybir.ActivationFunctionType
    ALU = mybir.AluOpType

    nc = bass.Bass()

    # ---------------- I/O ----------------
    band_in = nc.dram_tensor("band_in", [128, SH], f32, kind="ExternalInput")
    srcidx = nc.dram_tensor("srcidx", [128, KDEG * NBLK], i32, kind="ExternalInput")
    smallw = nc.dram_tensor("smallw", [128, 256], f32, kind="ExternalInput")
    btile_d = nc.dram_tensor("btile", [128, 16], f32, kind="ExternalInput")
    # backbone weights in bf16, small/phase-A weights in f32
    wnames = [
        ("i2", [128, 128], f32), ("sW", [64, 64], f32), ("w2", [128, 64], f32),
        ("m1W2", [128, 128], bf16), ("m2W2", [128, 4], bf16),
        ("m1cA", [128, 128], bf16), ("m1cB", [128, 128], bf16),
        ("m1cC", [64, 128], bf16),
        ("m1nA", [128, 128], bf16), ("m1nB", [128, 128], bf16),
        ("m1nC", [64, 128], bf16),
        ("m2cA", [128, 128], bf16), ("m2cB", [128, 128], bf16),
        ("m2cC", [64, 128], bf16),
        ("m2nA", [128, 128], bf16), ("m2nB", [128, 128], bf16),
        ("m2nC", [64, 128], bf16),
    ]
    wdram = {n: nc.dram_tensor(n, s, dt, kind="ExternalInput")
             for n, s, dt in wnames}
    y_out = nc.dram_tensor("y", [128, 1664], f32, kind="ExternalOutput")
    dbg = {}
    if debug:
        for nm, sh, dt in [("dxT0", [128, SH], bf16), ("dxT1", [128, SH], bf16),
                           ("dxT2", [64, SH], bf16), ("dhcb", [128, SH], f32),
                           ("dout1", [128, SH], f32), ("dtab", [1024, 128], bf16),
                           ("dmsg", [128, 512], bf16), ("dgd", [128, 512], bf16),
                           ("dacc2", [128, 1664], f32), ("dband", [128, SH], f32)]:
            dbg[nm] = nc.dram_tensor(nm, sh, dt, kind="ExternalOutput")

    # collective bounce buffers (bf16 neighbor tables)
    inb = [nc.dram_tensor(f"inb{v}", [SH, 128], bf16) for v in range(2)]
    outb = [
        nc.dram_tensor(f"outb{v}", [NPAD, 128], bf16, addr_space="Shared")
        for v in range(2)
    ]

    RG = [list(range(N_CORES))]

    with tile.TileContext(nc) as tc:
        with (
            tc.tile_pool(name="wpool", bufs=1) as wpool,
            tc.tile_pool(name="npool", bufs=1) as npool,
            tc.tile_pool(name="psA", bufs=4, space="PSUM") as psA,
            tc.tile_pool(name="psT", bufs=2, space="PSUM") as psT,
            tc.tile_pool(name="psO", bufs=2, space="PSUM") as psO,
        ):
            # ---------- persistent SBUF ----------
            band = npool.tile([128, SH], f32)       # xin@[0:6] oh@[32:42] t@[64:65] wall@[96:98]; recip->[64:65]
            xT0 = npool.tile([128, SH], bf16)       # init (conv1 x0)
            xT1 = npool.tile([128, SH], bf16)       # class(0:64) + sigma(64:128)
            xT2 = npool.tile([64, SH], bf16)        # wall_feat
            hcb = npool.tile([128, SH], f32)        # center term + b1 (per conv)
            acc = npool.tile([128, SH], f32)        # conv1 max accum
            accB = npool.tile([128, SH], bf16)      # relu(acc) = conv2's x0
            acc2v = npool.tile([128, 1664], f32)    # conv2 packed accum
            yv = npool.tile([128, 1664], f32)
            sidx = npool.tile([128, KDEG * NBLK], i32)

            smw = wpool.tile([128, 256], f32)
            bt = wpool.tile([128, 16], f32)
            ident = wpool.tile([128, 128], bf16)
            wt = {n: wpool.tile(s, dt, name=f"wt_{n}") for n, s, dt in wnames}

            nc.sync.dma_start(out=band[:], in_=band_in[:, :])
            nc.sync.dma_start(out=sidx[:], in_=srcidx[:, :])
            nc.sync.dma_start(out=smw[:], in_=smallw[:, :])
            nc.sync.dma_start(out=bt[:], in_=btile_d[:, :])
            for n, s, dt in wnames:
                nc.sync.dma_start(out=wt[n][:], in_=wdram[n][:, :])
            make_identity(nc, ident[:])

            def mm(out_ap, lhsT_ap, rhs_ap, start=True, stop=True,
                   tile_position=None):
                nc.tensor.matmul(
                    out_ap,
                    lhsT_ap,
                    rhs_ap,
                    start=start,
                    stop=stop,
                    tile_position=tile_position,
                )

            # bias column APs
            ib1 = bt[:, 0:1]
            ib2 = bt[:, 1:2]
            b_cs = bt[:, 2:3]        # [0:64]=0 (class), [64:128]=sb
            wb2 = bt[0:64, 4:5]
            m1b1 = bt[:, 5:6]
            m1b2 = bt[:, 6:7]
            m2b1 = bt[:, 7:8]
            m2b2v = bt[:, 8:9]       # b2 replicated at 32q+i

            for it_ in range(iters):
                # ---------- phase A: node features ----------
                with (
                    tc.tile_pool(name=f"stg{it_}", bufs=2) as stg,
                ):
                    for cs, cw in CHUNKS:
                        csl = slice(cs, cs + cw)
                        # init MLP
                        p1 = psA.tile([128, 512], f32, tag="pa")
                        mm(p1[:, :cw], smw[0:6, 0:128], band[0:6, csl])
                        i1s = stg.tile([128, 512], f32, tag="i1s")
                        nc.scalar.activation(i1s[:, :cw], p1[:, :cw], AF.Relu, bias=ib1)
                        p2 = psA.tile([128, 512], f32, tag="pa")
                        mm(p2[:, :cw], wt["i2"][:], i1s[:, :cw])
                        nc.scalar.activation(xT0[:, csl], p2[:, :cw], AF.Relu, bias=ib2)

                        # block-diag mm at base 64: rows 64..67 = [t, wall0,
                        # wall1, ones]. out [0:32]=t*w+256 (sin arg), [32:64]=
                        # t*w+0.25 (cos arg), [64:128]=wall_h1 pre-act. Range-
                        # reduce m = x - round(x) via i32 cast (round-nearest),
                        # then sin(2*pi*m) = sin(2*pi*x).
                        p3 = psA.tile([128, 512], f32, tag="pa")
                        mm(p3[:, :cw], smw[64:68, 0:128], band[64:68, csl])
                        gfpi = stg.tile([64, 512], i32, tag="gfpi")
                        nc.vector.tensor_copy(out=gfpi[0:64, :cw], in_=p3[0:64, :cw])
                        gfpf = stg.tile([64, 512], f32, tag="gfpf")
                        nc.vector.tensor_copy(out=gfpf[0:64, :cw], in_=gfpi[0:64, :cw])
                        gfpm = stg.tile([64, 512], f32, tag="gfpm")
                        nc.vector.scalar_tensor_tensor(
                            out=gfpm[0:64, :cw], in0=p3[0:64, :cw], scalar=0.0,
                            in1=gfpf[0:64, :cw], op0=ALU.bypass, op1=ALU.subtract)
                        gfps = stg.tile([64, 512], f32, tag="gfps")
                        nc.scalar.activation(gfps[0:64, :cw], gfpm[0:64, :cw], AF.Sin,
                                             scale=TWO_PI)
                        whs = stg.tile([128, 512], f32, tag="whs")
                        nc.scalar.activation(whs[64:128, :cw], p3[64:128, :cw], AF.Relu,
                                             bias=bt[64:128, 3:4])
                        # class (psum [0:64]) + sigma (psum [64:128])
                        p4 = psA.tile([128, 512], f32, tag="pa")
                        mm(p4[0:64, :cw], smw[32:42, 0:64], band[32:42, csl])
                        mm(p4[64:128, :cw], wt["sW"][:], gfps[:, :cw])
                        nc.scalar.activation(xT1[:, csl], p4[:, :cw], AF.Relu, bias=b_cs)

                        # wall MLP layer 2
                        p6 = psA.tile([128, 512], f32, tag="pa")
                        mm(p6[0:64, :cw], wt["w2"][64:128, :], whs[64:128, :cw])
                        nc.scalar.activation(xT2[:, csl], p6[0:64, :cw], AF.Relu, bias=wb2)

                        # std(t): exp -> (x-1)/(2 ln s) -> sqrt -> +eps -> recip
                        # all on partition 64 (lane-aligned)
                        sstd = stg.tile([128, 1024], f32, tag="sstd")
                        nc.scalar.activation(sstd[64:65, 0:cw], band[64:65, csl], AF.Exp,
                                             scale=2.0 * LOG_SIGMA)
                        nc.vector.tensor_scalar(
                            out=sstd[64:65, 512:512 + cw], in0=sstd[64:65, 0:cw],
                            scalar1=-1.0, scalar2=1.0 / (2.0 * LOG_SIGMA),
                            op0=ALU.add, op1=ALU.mult)
                        nc.scalar.activation(sstd[64:65, 0:cw],
                                             sstd[64:65, 512:512 + cw], AF.Sqrt)
                        nc.vector.tensor_scalar_add(
                            out=sstd[64:65, 512:512 + cw], in0=sstd[64:65, 0:cw],
                            scalar1=1e-7)
                        nc.vector.reciprocal(out=band[64:65, csl],
                                             in_=sstd[64:65, 512:512 + cw])

                nc.gpsimd.memset(acc[:], -3.0e38)
                nc.gpsimd.memset(acc2v[:], -3.0e38)

                # ---------- per-conv helpers ----------
                def x_mm(out_ap, wa, wb, wc, csl, cw, x0):
                    """out = W.T @ x for one chunk; x = [x0(128); xT1(128); xT2(64)]"""
                    mm(out_ap, wt[wa][:], x0[:, csl], start=True, stop=False)
                    mm(out_ap, wt[wb][:], xT1[:, csl], start=False, stop=False)
                    mm(out_ap, wt[wc][:], xT2[:, csl], start=False, stop=True)

                def build_table(conv, x0):
                    """hn = x @ Wb -> transpose -> DRAM shard -> AllGather."""
                    wa, wb, wc = (f"m{conv}nA", f"m{conv}nB", f"m{conv}nC")
                    with (
                        tc.tile_pool(name=f"hnst{conv}_{it_}", bufs=2) as hnp,
                        tc.tile_pool(name=f"rwst{conv}_{it_}", bufs=2) as rwp,
                    ):
                        for ci, (cs, cw) in enumerate(CHUNKS):
                            csl = slice(cs, cs + cw)
                            ph = psA.tile([128, 512], f32, tag="pa")
                            x_mm(ph[:, :cw], wa, wb, wc, csl, cw, x0)
                            hns = hnp.tile([128, 512], bf16, tag="hns")
                            nc.scalar.copy(hns[:, :cw], ph[:, :cw])
                            nb = cw // 128
                            pt = psT.tile([128, 512], bf16, tag="pt")
                            for b in range(nb):
                                nc.tensor.transpose(
                                    out=pt[:, 128 * b:128 * (b + 1)],
                                    in_=hns[:, 128 * b:128 * (b + 1)],
                                    identity=ident[:],
                                )
                            rws = rwp.tile([128, 512], bf16, tag="rws")
                            nc.vector.tensor_copy(out=rws[:, :cw], in_=pt[:, :cw])
                            dst_ap = inb[conv - 1][csl, :].rearrange(
                                "(b p) f -> p b f", p=128)
                            src_ap = rws[:].rearrange("p (b f) -> p b f", f=128)[:, :nb, :]
                            nc.sync.dma_start(out=dst_ap, in_=src_ap)
                    if "nocoll" not in ablate:
                        nc.gpsimd.collective_compute(
                            "AllGather", ALU.bypass, replica_groups=RG,
                            ins=[inb[conv - 1].ap().opt()],
                            outs=[outb[conv - 1].ap().opt()],
                        )

                def build_hcb(conv, x0):
                    wa, wb, wc = (f"m{conv}cA", f"m{conv}cB", f"m{conv}cC")
                    b1 = m1b1 if conv == 1 else m2b1
                    for cs, cw in CHUNKS:
                        csl = slice(cs, cs + cw)
                        ph = psA.tile([128, 512], f32, tag="pa")
                        x_mm(ph[:, :cw], wa, wb, wc, csl, cw, x0)
                        nc.scalar.activation(hcb[:, csl], ph[:, :cw], AF.Identity,
                                             bias=b1)

                def edge_pipeline(conv):
                    tab = outb[conv - 1]
                    gd_bufs = 8 if "deepbuf" in ablate else 3
                    with (
                        tc.tile_pool(name=f"gd{conv}_{it_}", bufs=gd_bufs) as gdp,
                        tc.tile_pool(name=f"ms{conv}_{it_}", bufs=3) as msp,
                    ):
                        for r in range(KDEG):
                            if conv == 1:
                                for ci, (cs, cw) in enumerate(CHUNKS):
                                    csl = slice(cs, cs + cw)
                                    nb = cw // 128
                                    m = _edge_msgs(tab, gdp, msp, r, ci, cs, cw, nb)
                                    if "noemm" in ablate or m is None:
                                        continue
                                    po = psO.tile([128, 512], f32, tag="po")
                                    mm(po[:, :cw], wt["m1W2"][:], m[:, :cw])
                                    nc.vector.scalar_tensor_tensor(
                                        out=acc[:, csl], in0=po[:, :cw],
                                        scalar=m1b2, in1=acc[:, csl],
                                        op0=ALU.add, op1=ALU.max)
                            else:
                                for G in range(4):
                                    ccs = list(range(4 * G, min(4 * G + 4, NCH)))
                                    po = psO.tile([128, 512], f32, tag="po")
                                    for qd, ci in enumerate(ccs):
                                        cs, cw = CHUNKS[ci]
                                        nb = cw // 128
                                        m = _edge_msgs(tab, gdp, msp, r, ci, cs, cw, nb)
                                        if "noemm" in ablate or m is None:
                                            continue
                                        mm(po[32 * qd:32 * qd + 4, :cw],
                                           wt["m2W2"][:], m[:, :cw],
                                           tile_position=(0, 32 * qd))
                                    if "noemm" in ablate or "gather_only" in ablate:
                                        continue
                                    if G < 3:
                                        nc.vector.scalar_tensor_tensor(
                                            out=acc2v[:, 512 * G:512 * (G + 1)],
                                            in0=po[:, :512], scalar=m2b2v,
                                            in1=acc2v[:, 512 * G:512 * (G + 1)],
                                            op0=ALU.add, op1=ALU.max)
                                    else:
                                        nc.vector.scalar_tensor_tensor(
                                            out=acc2v[0:4, 1536:1664],
                                            in0=po[0:4, :128], scalar=bt[0:4, 8:9],
                                            in1=acc2v[0:4, 1536:1664],
                                            op0=ALU.add, op1=ALU.max)

                dumped = {"done": False}

                def _edge_msgs(tab, gdp, msp, r, ci, cs, cw, nb):
                    gd = gdp.tile([128, 4, 128], bf16, tag="gd")
                    q0 = NBLK * r + 4 * ci
                    if "batch4" in ablate:
                        nc.gpsimd.indirect_dma_start(
                            out=gd[:, :nb, :],
                            out_offset=None,
                            in_=tab[:, :],
                            in_offset=bass.IndirectOffsetOnAxis(
                                ap=sidx[:, q0:q0 + nb], axis=0),
                        )
                    else:
                        for b in range(nb):
                            q = q0 + b
                            if "nogather" in ablate:
                                nc.sync.dma_start(
                                    out=gd[:, b, :],
                                    in_=tab[128 * b:128 * (b + 1), :])
                            else:
                                nc.gpsimd.indirect_dma_start(
                                    out=gd[:, b, :],
                                    out_offset=None,
                                    in_=tab[:, :],
                                    in_offset=bass.IndirectOffsetOnAxis(
                                        ap=sidx[:, q:q + 1], axis=0),
                                )
                    if "gather_only" in ablate:
                        return None
                    if "notrans" in ablate:
                        gflat = gd[:].rearrange("p a b -> p (a b)")
                        mf = msp.tile([128, 512], f32, tag="mf")
                        nc.vector.tensor_add(
                            out=mf[:, :cw], in0=gflat[:, :cw],
                            in1=hcb[:, cs:cs + cw])
                        m = msp.tile([128, 512], bf16, tag="m")
                        nc.scalar.activation(m[:, :cw], mf[:, :cw], AF.Relu)
                        return m
                    pt = psT.tile([128, 512], bf16, tag="pt")
                    for b in range(nb):
                        nc.tensor.transpose(
                            out=pt[:, 128 * b:128 * (b + 1)],
                            in_=gd[:, b, :],
                            identity=ident[:],
                        )
                    mf = msp.tile([128, 512], f32, tag="mf")
                    nc.vector.tensor_add(
                        out=mf[:, :cw], in0=pt[:, :cw],
                        in1=hcb[:, cs:cs + cw])
                    m = msp.tile([128, 512], bf16, tag="m")
                    nc.scalar.activation(m[:, :cw], mf[:, :cw], AF.Relu)
                    if debug and not dumped["done"]:
                        dumped["done"] = True
                        nc.sync.dma_start(
                            out=dbg["dgd"][:, :],
                            in_=gd[:].rearrange("p a b -> p (a b)"))
                        nc.sync.dma_start(out=dbg["dmsg"][:, :], in_=m[:])
                    return m

                # ---------- conv1 ----------
                build_table(1, xT0)
                build_hcb(1, xT0)
                if "noedge" not in ablate:
                    edge_pipeline(1)
                # out1 = relu(acc) -> bf16 (conv2's x0)
                for cs, cw in CHUNKS:
                    csl = slice(cs, cs + cw)
                    nc.scalar.activation(accB[:, csl], acc[:, csl], AF.Relu)

                if debug:
                    nc.sync.dma_start(out=dbg["dxT0"][:, :], in_=xT0[:])
                    nc.sync.dma_start(out=dbg["dxT1"][:, :], in_=xT1[:])
                    nc.sync.dma_start(out=dbg["dxT2"][:, :], in_=xT2[:])
                    nc.sync.dma_start(out=dbg["dhcb"][:, :], in_=hcb[:])
                    nc.sync.dma_start(out=dbg["dout1"][:, :], in_=acc[:])
                    nc.sync.dma_start(out=dbg["dtab"][:, :],
                                      in_=outb[0][25000:26024, :])
                    nc.sync.dma_start(out=dbg["dband"][:, :], in_=band[:])

                # ---------- conv2 ----------
                build_table(2, accB)
                build_hcb(2, accB)
                if "noedge" not in ablate:
                    edge_pipeline(2)

                if debug:
                    nc.sync.dma_start(out=dbg["dacc2"][:, :], in_=acc2v[:])

                # ---------- final scale: y = acc2 * recip ----------
                for G in range(4):
                    ccs = list(range(4 * G, min(4 * G + 4, NCH)))
                    pr = psA.tile([128, 512], f32, tag="pa")
                    gw = 512 if G < 3 else 128
                    for qd, ci in enumerate(ccs):
                        cs, cw = CHUNKS[ci]
                        mm(pr[32 * qd:32 * qd + 4, :cw],
                           smw[64:65, 132:136], band[64:65, cs:cs + cw],
                           tile_position=(64, 32 * qd))
                    gcol = slice(512 * G, 512 * G + gw)
                    if G < 3:
                        nc.vector.tensor_tensor(
                            out=yv[:, gcol], in0=acc2v[:, gcol], in1=pr[:, :gw],
                            op=ALU.mult)
                    else:
                        nc.vector.tensor_tensor(
                            out=yv[0:4, gcol], in0=acc2v[0:4, gcol],
                            in1=pr[0:4, :gw], op=ALU.mult)
                nc.sync.dma_start(out=y_out[:, :], in_=yv[:])

    _split_multi_waits(nc, mybir)
    return nc


def _host_prep(inputs):
    """Build per-core input maps from full inputs."""
    t = np.asarray(inputs["t"], np.float32).reshape(N)
    obj_x = np.asarray(inputs["obj_x"], np.float32)
    obj_geo = np.asarray(inputs["obj_geo"], np.float32)
    wall = np.asarray(inputs["wall"], np.float32)
    category = np.asarray(inputs["category"]).astype(np.int64)
    batch_idx = np.asarray(inputs["batch_idx"]).astype(np.int64)
    src = np.asarray(inputs["src"]).astype(np.int64)
    dst = np.asarray(inputs["dst"]).astype(np.int64)

    # edge slots: slot (r, n) holds the r-th incoming edge of node n
    if np.array_equal(dst, np.tile(np.arange(N, dtype=dst.dtype), E // N)):
        src_slots = src.reshape(KDEG, N)
    else:
        order = np.argsort(dst, kind="stable")
        counts = np.bincount(dst, minlength=N)
        assert (counts == KDEG).all(), "kernel requires uniform in-degree 10"
        src_slots = np.empty((KDEG, N), np.int64)
        srt = src[order].reshape(N, KDEG)
        src_slots[:, :] = srt.T

    wall_pn = wall[batch_idx]  # [N, 2]

    # weights (shared)
    def f32c(x):
        return np.ascontiguousarray(np.asarray(x, np.float32))

    smallw = np.zeros((128, 256), np.float32)
    smallw[0:6, 0:128] = f32c(inputs["i1"])
    smallw[32:42, 0:64] = f32c(inputs["embed_W"])
    gw = f32c(inputs["gfp_W"]).reshape(32)
    smallw[64, 0:32] = gw
    smallw[64, 32:64] = gw
    smallw[65:67, 64:128] = f32c(inputs["w1"])   # wall layer 1 block
    smallw[67, 32:64] = 0.25      # cos = sin(2*pi*(x + 1/4))
    smallw[64, 132:136] = 1.0  # ones4 for recip broadcast

    btile = np.zeros((128, 16), np.float32)
    btile[:, 0] = f32c(inputs["ib1"])
    btile[:, 1] = f32c(inputs["ib2"])
    btile[64:128, 2] = f32c(inputs["sb"])
    btile[64:128, 3] = f32c(inputs["wb1"])
    btile[0:64, 4] = f32c(inputs["wb2"])
    btile[:, 5] = f32c(inputs["m1b1"])
    btile[:, 6] = f32c(inputs["m1b2"])
    btile[:, 7] = f32c(inputs["m2b1"])
    b2 = f32c(inputs["m2b2"])
    for q in range(4):
        btile[32 * q:32 * q + 4, 8] = b2
    btile[:, 9] = np.float32(np.pi / 2.0)
    btile[:, 10] = np.float32(-np.pi)

    m1W1 = f32c(inputs["m1W1"])
    m2W1 = f32c(inputs["m2W1"])
    m1c = m1W1[:320] - m1W1[320:]
    m1n = m1W1[320:]
    m2c = m2W1[:320] - m2W1[320:]
    m2n = m2W1[320:]

    import ml_dtypes
    bf = ml_dtypes.bfloat16
    wmap = {
        "i2": f32c(inputs["i2"]), "sW": f32c(inputs["sW"]),
        "w2": np.concatenate(
            [np.zeros((64, 64), np.float32), f32c(inputs["w2"])], axis=0),
        "m1W2": f32c(inputs["m1W2"]).astype(bf),
        "m2W2": f32c(inputs["m2W2"]).astype(bf),
        "m1cA": m1c[0:128].astype(bf), "m1cB": m1c[128:256].astype(bf),
        "m1cC": m1c[256:320].astype(bf),
        "m1nA": m1n[0:128].astype(bf), "m1nB": m1n[128:256].astype(bf),
        "m1nC": m1n[256:320].astype(bf),
        "m2cA": m2c[0:128].astype(bf), "m2cB": m2c[128:256].astype(bf),
        "m2cC": m2c[256:320].astype(bf),
        "m2nA": m2n[0:128].astype(bf), "m2nB": m2n[128:256].astype(bf),
        "m2nC": m2n[256:320].astype(bf),
    }
    wmap = {k: np.ascontiguousarray(v) for k, v in wmap.items()}

    in_maps = []
    for c in range(N_CORES):
        n0 = c * SH
        nreal = min(max(N - n0, 0), SH)
        band = np.zeros((128, SH), np.float32)
        band[0:4, :nreal] = obj_x[n0:n0 + nreal].T
        band[4:6, :nreal] = obj_geo[n0:n0 + nreal].T
        cat = category[n0:n0 + nreal]
        band[32 + cat, np.arange(nreal)] = 1.0  # one-hot
        band[64, :nreal] = t[n0:n0 + nreal]
        band[64, nreal:] = 1.0
        band[65:67, :nreal] = wall_pn[n0:n0 + nreal].T
        band[67, :] = 1.0

        sidx = np.zeros((128, KDEG * NBLK), np.int32)
        sl = src_slots[:, n0:n0 + nreal]  # [10, nreal]
        sfull = np.zeros((KDEG, SH), np.int64)
        sfull[:, :nreal] = sl
        # column q = 49*r + j holds nodes 128j..128j+127 of round r
        sidx[:, :] = (
            sfull.reshape(KDEG, NBLK, 128).transpose(2, 0, 1).reshape(
                128, KDEG * NBLK)
        ).astype(np.int32)

        im = {
            "band_in": band,
            "srcidx": sidx,
            "smallw": smallw,
            "btile": btile,
        }
        im.update(wmap)
        in_maps.append(im)
    return in_maps


def _unshard(results):
    out = np.empty((NPAD, 4), np.float32)
    for c in range(N_CORES):
        yv = results[c]["y"]  # [128, 1664]
        for cc in range(NCH):
            G, q = cc // 4, cc % 4
            cs, cw = CHUNKS[cc]
            out[c * SH + cs: c * SH + cs + cw, :] = (
                yv[32 * q:32 * q + 4, 512 * G:512 * G + cw].T
            )
    return out[:N]


def _make_runner(nc):
    """Persistent jitted PJRT executable for `nc` (8-core SPMD). Returns
    (fn, stage, out_names, out_shapes): call fn(*stage(in_maps)) and reshape.
    Avoids per-call retrace + NEFF reload of run_bass_kernel_spmd."""
    import jax
    from jax.sharding import Mesh, PartitionSpec, NamedSharding
    from jax.experimental.shard_map import shard_map
    from concourse import mybir
    from concourse.bass2jax import (_bass_exec_p, install_neuronx_cc_hook,
                                    partition_id_tensor)

    install_neuronx_cc_hook()
    partition_name = (nc.partition_id_tensor.name
                      if nc.partition_id_tensor else None)
    in_names, out_names, out_avals, zero_outs = [], [], [], []
    for alloc in nc.m.functions[0].allocations:
        if not isinstance(alloc, mybir.MemoryLocationSet):
            continue
        name = alloc.memorylocations[0].name
        if alloc.kind == "ExternalInput":
            if name != partition_name:
                in_names.append(name)
        elif alloc.kind == "ExternalOutput":
            out_names.append(name)
            shape = tuple(alloc.tensor_shape)
            dtype = mybir.dt.np(alloc.dtype)
            out_avals.append(jax.core.ShapedArray(shape, dtype))
            zero_outs.append(np.zeros(shape, dtype))
    n_params = len(in_names)
    all_names = in_names + out_names
    if partition_name is not None:
        all_names = all_names + [partition_name]

    def _body(*args):
        operands = list(args)
        if partition_name is not None:
            operands.append(partition_id_tensor())
        outs = _bass_exec_p.bind(
            *operands,
            out_avals=tuple(out_avals),
            in_names=tuple(all_names),
            out_names=tuple(out_names),
            lowering_input_output_aliases=(),
            sim_require_finite=True,
            sim_require_nnan=True,
            nc=nc,
        )
        return tuple(outs)

    devices = jax.devices()[:N_CORES]
    mesh = Mesh(np.asarray(devices), ("core",))
    n_out = len(out_names)
    fn = jax.jit(
        shard_map(_body, mesh=mesh,
                  in_specs=(PartitionSpec("core"),) * (n_params + n_out),
                  out_specs=(PartitionSpec("core"),) * n_out,
                  check_rep=False),
        keep_unused=True,
    )
    sharding = NamedSharding(mesh, PartitionSpec("core"))

    def stage(in_maps):
        concat_in = [
            np.concatenate([np.asarray(in_maps[c][nm])
                            for c in range(N_CORES)], axis=0)
            for nm in in_names
        ]
        concat_zeros = [
            np.zeros((N_CORES * z.shape[0], *z.shape[1:]), z.dtype)
            for z in zero_outs
        ]
        import jax as _jax
        return [_jax.device_put(x, sharding)
                for x in concat_in + concat_zeros]

    return fn, stage, out_names, [a.shape for a in out_avals]


def _run_spmd(nc, in_maps):
    """Run via cached persistent runner; returns per-core dict of outputs."""
    import jax
    key = id(nc)
    if _CACHE.get("runner_key") != key:
        _CACHE["runner"] = _make_runner(nc)
        _CACHE["runner_key"] = key
    fn, stage, out_names, out_shapes = _CACHE["runner"]
    dev_in = stage(in_maps)
    outs = jax.block_until_ready(fn(*dev_in))
    res = []
    for c in range(N_CORES):
        d = {}
        for i, nm in enumerate(out_names):
            g = np.asarray(outs[i])
            d[nm] = g.reshape(N_CORES, *out_shapes[i])[c]
        res.append(d)
    return res


def kernel(**inputs) -> np.ndarray:
    import time as _time

    if "nc" not in _CACHE:
        _CACHE["nc"] = _build()
    nc = _CACHE["nc"]
    in_maps = _host_prep(inputs)
    last_err = None
    for attempt in range(4):
        try:
            results = _run_spmd(nc, in_maps)
            break
        except Exception as e:  # transient NRT device wedge recovers on retry
            last_err = e
            _time.sleep(10 * (attempt + 1))
    else:
        # final fallback: the stock SPMD path
        from concourse.bass_utils import run_bass_kernel_spmd
        try:
            res = run_bass_kernel_spmd(nc, in_maps,
                                       core_ids=list(range(N_CORES)))
            results = res.results
        except Exception:
            raise last_err
    return _unshard(results)


import concourse.bass as bass  # noqa: E402  (used inside _build closures)

